# revision 3
# baseline (speedup 1.0000x reference)
"""Trainium2 Bass kernel for nn_DQA_89077621719347 (dense_cnn, 8 cores).

Math (per batch b, channel c):
  feat_ave = mean_{h,w} feat                      # (b, c)
  CMA(feat_ave, deg) -> cma; emb = gamma*cma + deg
  kern = (lrelu(emb @ k_w1.T) @ k_w2.T)           # per-(b,c) 3x3 kernel
  z    = lrelu(depthwise3x3(feat, kern))
  out  = conv_w @ z + conv_b + feat * sigmoid(lrelu(deg@ca_w1.T)@ca_w2.T)

Sharding: data-parallel over batch, 2 batches/core -> 128 partitions=(b,c).

Engine split of the depthwise conv (per group of 2 output rows):
  - PE:  6 side taps (kx!=1) as diagonal-weight bf16 matmuls into PSUM pd,
         then 1x1-conv (block-diag conv_w) + diag(att)*feat residual matmul
         accumulated into PSUM po.
  - DVE: 3 center taps (kx==1, 4B-aligned windows) as fused
         scalar_tensor_tensor multiply-adds, then join acc+pd.
  - Scalar: lrelu via AF.Lrelu(alpha), final out = po + conv_b via
         Identity+bias, both single ops.
Feat is cached in SBUF as bf16 with 2-col left/right pads (pw=260 even so
center-tap windows are 4B aligned) and a zero row above/below.
"""
import contextlib

import numpy as np

import concourse.bass as bass
import concourse.bacc as bacc
import concourse.tile as tile
import concourse.mybir as mybir
from concourse.masks import make_identity

f32 = mybir.dt.float32
bf16 = mybir.dt.bfloat16
AF = mybir.ActivationFunctionType
OP = mybir.AluOpType

B, C, H, W = 16, 64, 256, 256
NCORES = 8
BPC = B // NCORES          # batches per core
P = BPC * C                # 128 partitions

SIDE = [0, 2, 3, 5, 6, 8]  # taps with kx != 1 (PE)
CENT = [1, 4, 7]           # taps with kx == 1 (DVE)


def build_nc(h=H, w=W, loop_reps=1):
    """Build the per-core SPMD Bass module (shapes [BPC,C,h,w]).

    loop_reps>1 wraps the computation in a hardware For_i loop — used only
    for timing (per-iteration time = wall-clock delta / extra reps)."""
    pw = w + 4                 # padded row width: 2 left pad cols, 2 right
    ph = h + 2                 # zero row above and below
    cn = ph * pw
    npx = h * w
    n_groups = h // 2          # 2 output rows per group
    lr = min(8, h)             # image rows per pass-1 load slab
    n_slabs = h // lr

    nc = bacc.Bacc(trn_type="TRN2")

    feat = nc.dram_tensor("feat", [BPC, C, h, w], f32, kind="ExternalInput")
    deg = nc.dram_tensor("deg", [BPC, C], f32, kind="ExternalInput")
    wq = nc.dram_tensor("wq", [C, C], f32, kind="ExternalInput")
    bq = nc.dram_tensor("bq", [C], f32, kind="ExternalInput")
    wk = nc.dram_tensor("wk", [C, C], f32, kind="ExternalInput")
    bk = nc.dram_tensor("bk", [C], f32, kind="ExternalInput")
    wv = nc.dram_tensor("wv", [C, C], f32, kind="ExternalInput")
    bv = nc.dram_tensor("bv", [C], f32, kind="ExternalInput")
    gamma = nc.dram_tensor("gamma", [1], f32, kind="ExternalInput")
    k_w1 = nc.dram_tensor("k_w1", [C, C], f32, kind="ExternalInput")
    k_w2 = nc.dram_tensor("k_w2", [C * 9, C], f32, kind="ExternalInput")
    conv_w = nc.dram_tensor("conv_w", [C, C], f32, kind="ExternalInput")
    conv_b = nc.dram_tensor("conv_b", [C], f32, kind="ExternalInput")
    ca_w1 = nc.dram_tensor("ca_w1", [C // 8, C], f32, kind="ExternalInput")
    ca_w2 = nc.dram_tensor("ca_w2", [C, C // 8], f32, kind="ExternalInput")
    out = nc.dram_tensor("out", [BPC, C, h, w], f32, kind="ExternalOutput")

    featv = feat[:, :, :, :].rearrange("b c h w -> (b c) (h w)")
    outv = out[:, :, :, :].rearrange("b c h w -> (b c) (h w)")

    with tile.TileContext(nc) as tc, contextlib.ExitStack() as ctx:
        sing = ctx.enter_context(tc.tile_pool(name="sing", bufs=1))
        work = ctx.enter_context(tc.tile_pool(name="work", bufs=3))
        dr = ctx.enter_context(tc.tile_pool(name="dr", bufs=1, space="DRAM"))
        ps_v = ctx.enter_context(tc.tile_pool(name="ps_v", bufs=1, space="PSUM"))
        ps_d = ctx.enter_context(tc.tile_pool(name="ps_d", bufs=3, space="PSUM"))
        ps_o = ctx.enter_context(tc.tile_pool(name="ps_o", bufs=2, space="PSUM"))
        stgp = ctx.enter_context(tc.tile_pool(name="stgp", bufs=2))

        def emit():
            # ------------- constants / weight prep (no feat dependency) -----
            ident_b = sing.tile([128, 128], bf16)
            make_identity(nc, ident_b[:, :])

            def load_T(src_dram, rows, cols, name):
                """Transposed load: DRAM [rows, cols] -> SBUF [cols, rows]
                via a strided AP (tiny tensors; cost irrelevant)."""
                t = sing.tile([cols, rows], f32, tag=f"T{name}")
                ap = bass.AP(tensor=src_dram[:, :].tensor, offset=0,
                             ap=[[1, cols], [cols, rows]])
                nc.sync.dma_start(out=t[:, :], in_=ap)
                return t

            def blkdiag(tsb, rows, cols, dtype=f32, name=""):
                """[128,128] block-diagonal from tsb ([rows, cols]): one block
                per batch at (b*64, b*64)."""
                blk = sing.tile([128, 128], dtype, tag=f"blk{name}")
                nc.gpsimd.memset(blk[:, :], 0.0)
                nc.vector.tensor_copy(blk[0:rows, 0:cols], tsb[:, :])
                nc.sync.dma_start(out=blk[64:64 + rows, 64:64 + cols],
                                  in_=tsb[:, :])
                return blk

            wqT = load_T(wq, 64, 64, "wq")
            wkT = load_T(wk, 64, 64, "wk")
            wvT = load_T(wv, 64, 64, "wv")
            k_w1T = load_T(k_w1, 64, 64, "kw1")
            conv_wT = load_T(conv_w, 64, 64, "cw")
            ca_w1T = load_T(ca_w1, 8, 64, "ca1")      # [64, 8]
            ca_w2T = load_T(ca_w2, 64, 8, "ca2")      # [8, 64]

            BQ = blkdiag(wqT, 64, 64, name="q")
            BK = blkdiag(wkT, 64, 64, name="k")
            BV = blkdiag(wvT, 64, 64, name="v")
            BW1 = blkdiag(k_w1T, 64, 64, name="w1")
            BA1 = blkdiag(ca_w1T, 64, 8, name="a1")
            BA2 = blkdiag(ca_w2T, 8, 64, name="a2")
            conv_wTb = sing.tile([64, 64], bf16)
            nc.vector.tensor_copy(conv_wTb[:, :], conv_wT[:, :])
            CB = blkdiag(conv_wTb, 64, 64, dtype=bf16, name="cw")

            # per-tap kernel-predictor weights: BK2[t][(b,j),(b,c)] =
            # k_w2[c*9+t, j] so that k_tap[:, t] = BK2[t].T @ hid
            BK2 = []
            for t in range(9):
                w2t = sing.tile([64, 64], f32, tag=f"w2T{t}")
                ap = bass.AP(tensor=k_w2[:, :].tensor, offset=t * 64,
                             ap=[[1, 64], [9 * 64, 64]])
                nc.sync.dma_start(out=w2t[:, :], in_=ap)
                BK2.append(blkdiag(w2t, 64, 64, name=f"k2_{t}"))

            # per-partition vectors
            def bcast_c(src, name):
                t = sing.tile([128, 1], f32, tag=f"pc{name}")
                ap = bass.AP(tensor=src[:].tensor, offset=0,
                             ap=[[0, BPC], [1, C]])
                nc.gpsimd.dma_start(out=t[:, 0:1], in_=ap)
                return t

            bq_pc = bcast_c(bq, "bq")
            bk_pc = bcast_c(bk, "bk")
            bv_pc = bcast_c(bv, "bv")
            conv_b_pc = bcast_c(conv_b, "cb")
            gamma_pc = sing.tile([128, 1], f32)
            nc.gpsimd.dma_start(
                out=gamma_pc[:, 0:1],
                in_=bass.AP(tensor=gamma[:].tensor, offset=0,
                            ap=[[0, 128], [1, 1]]))
            deg_pc = sing.tile([128, 1], f32)
            nc.sync.dma_start(out=deg_pc[:, 0:1],
                              in_=deg[:, :].rearrange("b c -> (b c)")
                              .rearrange("(p one) -> p one", one=1))

            def vec_mm(blk_w, rhs_pc, name):
                """[128,1] = blkdiag.T @ rhs (tiny matmul), result in PSUM."""
                p = ps_v.tile([128, 1], f32, tag="vec")
                nc.tensor.matmul(p[:, 0:1], blk_w[:, :], rhs_pc[:, 0:1],
                                 start=True, stop=True)
                return p

            def lrelu_vec(psum_in, name):
                """lrelu on a [128,1] psum -> sbuf f32 via AF.Lrelu."""
                o = sing.tile([128, 1], f32, tag=f"lro{name}")
                nc.scalar.activation(out=o[:, 0:1], in_=psum_in[:, 0:1],
                                     func=AF.Prelu, bias=0.0, scale=1.0,
                                     alpha=0.1)
                return o

            # --------- channel attention (depends only on deg) ---------------
            a0 = vec_mm(BA1, deg_pc, "a0")
            t_pr = lrelu_vec(a0, "a0")
            a1 = vec_mm(BA2, t_pr, "a1")
            att_pc = sing.tile([128, 1], f32)
            nc.scalar.activation(out=att_pc[:, 0:1], in_=a1[:, 0:1],
                                 func=AF.Sigmoid, bias=0.0, scale=1.0)
            # diag(att) as bf16 [128,128] for the residual matmul
            ATTD = sing.tile([128, 128], bf16)
            nc.vector.tensor_scalar_mul(ATTD[:, :], ident_b[:, :],
                                        att_pc[:, 0:1])

            # --------- kk / v (depend only on deg) ---------------------------
            kk0 = vec_mm(BK, deg_pc, "kk")
            kk_pi = sing.tile([128, 1], f32)
            nc.scalar.activation(out=kk_pi[:, 0:1], in_=kk0[:, 0:1],
                                 func=AF.Identity, bias=bk_pc[:, 0:1],
                                 scale=1.0)
            v0 = vec_mm(BV, deg_pc, "v")
            v_pi = sing.tile([128, 1], f32)
            nc.scalar.activation(out=v_pi[:, 0:1], in_=v0[:, 0:1],
                                 func=AF.Identity, bias=bv_pc[:, 0:1],
                                 scale=1.0)

            def rep64(src_pc, name):
                """[128,1] (p=(b,i)) -> [128,64] tile whose row (b,j) is
                src[b*64 : b*64+64] (replicated across j)."""
                d = dr.tile([128], f32, tag=f"dr{name}")
                nc.sync.dma_start(out=d[:], in_=src_pc[:, 0:1])
                rep = sing.tile([128, 64], f32, tag=f"rep{name}")
                ap = bass.AP(tensor=d[:].tensor, offset=d[:].offset,
                             ap=[[64, BPC], [0, C], [1, C]])
                nc.sync.dma_start(out=rep[:, :], in_=ap)
                return rep

            kk_rep = rep64(kk_pi, "kk")
            v_rep = rep64(v_pi, "v")

            # ------------- pass 1: load feat -> bf16 cache + row sums --------
            cache = sing.tile([128, cn], bf16)
            # zero pads: top row, bottom row, 2 left + 2 right cols per row
            nc.gpsimd.memset(cache[:, 0:pw], 0.0)
            nc.gpsimd.memset(cache[:, (ph - 1) * pw:cn], 0.0)
            mid = cache[:, pw:(ph - 1) * pw].rearrange("p (h w) -> p h w",
                                                       w=pw)
            nc.gpsimd.memset(mid[:, :, 0:2], 0.0)
            nc.gpsimd.memset(mid[:, :, w + 2:w + 4], 0.0)

            partials = sing.tile([128, n_slabs], f32)
            for i in range(n_slabs):
                base = (i * lr + 1) * pw
                dst = cache[:, base:base + lr * pw] \
                    .rearrange("p (h w) -> p h w", w=pw)[:, :, 2:2 + w]
                src = featv[:, i * lr * w:(i + 1) * lr * w]
                stg = stgp.tile([128, lr * w], f32, tag="stg")
                nc.sync.dma_start(out=stg[:, :], in_=src)
                nc.scalar.activation(
                    out=dst, in_=stg[:, :].rearrange("p (h w) -> p h w", w=w),
                    func=AF.Copy, bias=0.0, scale=1.0)
                nc.vector.reduce_sum(partials[:, i:i + 1], stg[:, :],
                                     axis=mybir.AxisListType.X)

            # Hard barrier: every cache/staging write must land before the
            # CMA chain and tap matmuls below may start.  (Without it the
            # Tile schedule is nondeterministic across builds and some
            # schedules produced wrong output on HW.)
            tc.strict_bb_all_engine_barrier()

            feat_ave = sing.tile([128, 1], f32)
            nc.vector.reduce_sum(feat_ave[:, 0:1], partials[:, :],
                                 axis=mybir.AxisListType.X)
            nc.vector.tensor_scalar_mul(feat_ave[:, 0:1], feat_ave[:, 0:1],
                                        1.0 / npx)

            # ------------- CMA + kernel-predictor MLP ------------------------
            q0 = vec_mm(BQ, feat_ave, "q")
            q_pj = sing.tile([128, 1], f32)
            nc.scalar.activation(out=q_pj[:, 0:1], in_=q0[:, 0:1],
                                 func=AF.Identity, bias=bq_pc[:, 0:1],
                                 scale=1.0)

            # softmax without max-subtraction: |energy| is O(1) by construction
            energy = sing.tile([128, C], f32)
            nc.vector.tensor_scalar_mul(energy[:, :], kk_rep[:, :],
                                        q_pj[:, 0:1])
            ee = sing.tile([128, C], f32)
            nc.scalar.activation(out=ee[:, :], in_=energy[:, :], func=AF.Exp,
                                 bias=0.0, scale=1.0)
            es = sing.tile([128, 1], f32)
            nc.vector.reduce_sum(es[:, 0:1], ee[:, :],
                                 axis=mybir.AxisListType.X)
            erc = sing.tile([128, 1], f32)
            nc.vector.reciprocal(erc[:, 0:1], es[:, 0:1])
            attn = sing.tile([128, C], f32)
            nc.vector.tensor_scalar_mul(attn[:, :], ee[:, :], erc[:, 0:1])
            prod = sing.tile([128, C], f32)
            nc.vector.tensor_mul(prod[:, :], attn[:, :], v_rep[:, :])
            cma = sing.tile([128, 1], f32)
            nc.vector.reduce_sum(cma[:, 0:1], prod[:, :],
                                 axis=mybir.AxisListType.X)
            emb = sing.tile([128, 1], f32)
            nc.vector.scalar_tensor_tensor(emb[:, 0:1], cma[:, 0:1],
                                           gamma_pc[:, 0:1], deg_pc[:, 0:1],
                                           op0=OP.mult, op1=OP.add)

            hid0 = vec_mm(BW1, emb, "hid")
            hid_pc = lrelu_vec(hid0, "hid")

            # k_tap[:, t] = BK2[t].T @ hid  -> all 9 taps into one PSUM bank
            ktp = ps_v.tile([128, 9], f32, tag="ktap")
            for t in range(9):
                nc.tensor.matmul(ktp[:, t:t + 1], BK2[t][:, :],
                                 hid_pc[:, 0:1], start=True, stop=True)
            k_tap = sing.tile([128, 9], f32)
            nc.scalar.copy(k_tap[:, :], ktp[:, :])

            diags = {}
            for t in SIDE:
                dg = sing.tile([128, 128], bf16, tag=f"diag{t}")
                nc.vector.tensor_scalar_mul(dg[:, :], ident_b[:, :],
                                            k_tap[:, t:t + 1])
                diags[t] = dg

            # ------------- main loop: 2 output rows per group ----------------
            cap = cache[:, :]

            def win(r0, idx):
                # [128, 2, w] strided window for tap idx at output rows
                # (r0, r0+1); data cols start at col 2.
                ky, kx = idx // 3, idx % 3
                base = (r0 + ky - 1) * pw + 2 + (kx - 1)
                return bass.AP(tensor=cap.tensor, offset=cap.offset + base,
                               ap=[list(cap.ap[0]), [pw, 2], [1, w]])

            for g in range(n_groups):
                r0 = 2 * g + 1
                # --- PE: 6 side taps into pd ---
                pd = ps_d.tile([128, 2 * w], f32, tag="pd")
                for j, idx in enumerate(SIDE):
                    nc.tensor.matmul(pd[:, :], diags[idx][:, :], win(r0, idx),
                                     start=(j == 0), stop=(j == len(SIDE) - 1))
                # --- DVE: 3 center taps fused mults/adds ---
                a0t = work.tile([128, 2, w], bf16, tag="a0t")
                nc.vector.tensor_scalar_mul(a0t[:, :, :], win(r0, CENT[0]),
                                            k_tap[:, CENT[0]:CENT[0] + 1])
                a1t = work.tile([128, 2, w], bf16, tag="a1t")
                nc.vector.scalar_tensor_tensor(
                    a1t[:, :, :], win(r0, CENT[1]),
                    k_tap[:, CENT[1]:CENT[1] + 1], a0t[:, :, :],
                    op0=OP.mult, op1=OP.add)
                a2t = work.tile([128, 2, w], bf16, tag="a2t")
                nc.vector.scalar_tensor_tensor(
                    a2t[:, :, :], win(r0, CENT[2]),
                    k_tap[:, CENT[2]:CENT[2] + 1], a1t[:, :, :],
                    op0=OP.mult, op1=OP.add)
                # --- DVE: join accumulator with PE psum ---
                y_pre = work.tile([128, 2 * w], bf16, tag="y_pre")
                nc.vector.tensor_add(y_pre[:, :],
                                     a2t[:, :, :].rearrange("p a b -> p (a b)"),
                                     pd[:, :])
                # --- Scalar: lrelu in one op ---
                y = work.tile([128, 2 * w], bf16, tag="y")
                nc.scalar.activation(out=y[:, :], in_=y_pre[:, :],
                                     func=AF.Prelu, bias=0.0, scale=1.0,
                                     alpha=0.1)
                # --- PE: 1x1 conv + att*feat residual into po ---
                po = ps_o.tile([128, 2 * w], f32, tag="po")
                nc.tensor.matmul(po[:, :], CB[:, :], y[:, :],
                                 start=True, stop=False)
                nc.tensor.matmul(po[:, :], ATTD[:, :], win(r0, 4),
                                 start=False, stop=True)
                # --- Scalar: add conv_b, emit f32 ---
                out_s = work.tile([128, 2 * w], f32, tag="out_s")
                nc.scalar.activation(out=out_s[:, :], in_=po[:, :],
                                     func=AF.Identity, bias=conv_b_pc[:, 0:1],
                                     scale=1.0)
                nc.sync.dma_start(out=outv[:, g * 2 * w:(g + 1) * 2 * w],
                                  in_=out_s[:, :])

        if loop_reps > 1:
            with tc.For_i(0, loop_reps, 1):
                emit()
        else:
            emit()

    nc.finalize()
    return nc


_NC_CACHE = {}


def _get_nc(h, w):
    if (h, w) not in _NC_CACHE:
        _NC_CACHE[(h, w)] = build_nc(h, w)
    return _NC_CACHE[(h, w)]


def kernel(**inputs):
    from concourse.bass_utils import run_bass_kernel_spmd

    feat = np.ascontiguousarray(inputs["feat"], dtype=np.float32)
    deg = np.ascontiguousarray(inputs["deg"], dtype=np.float32)
    b, c, h, w = feat.shape
    nc = _get_nc(h, w)

    shared = {k: np.ascontiguousarray(np.asarray(v), dtype=np.float32)
              for k, v in inputs.items() if k not in ("feat", "deg")}
    in_maps = []
    for k in range(NCORES):
        m = dict(shared)
        m["feat"] = feat[k * BPC:(k + 1) * BPC]
        m["deg"] = deg[k * BPC:(k + 1) * BPC]
        in_maps.append(m)

    res = run_bass_kernel_spmd(nc, in_maps, core_ids=list(range(NCORES)))
    return np.concatenate([r["out"] for r in res.results], axis=0)


# revision 5
# speedup vs baseline: 1.4524x; 1.4524x over previous
"""Trainium2 Bass kernel for nn_DQA_89077621719347 (dense_cnn, 8 cores).

Math (per batch b, channel c):
  feat_ave = mean_{h,w} feat                      # (b, c)
  CMA(feat_ave, deg) -> cma; emb = gamma*cma + deg
  kern = (lrelu(emb @ k_w1.T) @ k_w2.T)           # per-(b,c) 3x3 kernel
  z    = lrelu(depthwise3x3(feat, kern))
  out  = conv_w @ z + conv_b + feat * sigmoid(lrelu(deg@ca_w1.T)@ca_w2.T)

Sharding: data-parallel over batch, 2 batches/core -> 128 partitions=(b,c).

Engine split per group (2 output rows, 512 px):
  - DVE:  3 center taps (kx==1, 4B-aligned windows) as 4x-mode
          tensor_scalar muls + two 2x-mode adds -> acc (bf16).
  - PE:   6 side taps as diagonal-weight bf16 matmuls into PSUM pd, an
          identity matmul accumulating acc into pd (join), the 1x1-conv
          (block-diag conv_w) and a diag(att) residual matmul into PSUM po.
  - Scalar: y = Prelu(pd) in one op; out = po + conv_b via Identity+bias.

Software pipelining: feat lives in four bf16 quarter caches (66 padded rows
each, pw=260: 2 pad cols per side keeps center windows 4B aligned).  A
prologue fills them once.  Each For_i body runs the CMA chain + main loop
on the current caches while textually interleaved refill ops (DMA -> scalar
Copy -> DVE row-sum) rewrite quarter q right after its groups are consumed,
so the next iteration's load hides under this iteration's compute.
"""
import contextlib

import numpy as np

import concourse.bass as bass
import concourse.bacc as bacc
import concourse.tile as tile
import concourse.mybir as mybir
from concourse.masks import make_identity

f32 = mybir.dt.float32
bf16 = mybir.dt.bfloat16
AF = mybir.ActivationFunctionType
OP = mybir.AluOpType

B, C, H, W = 16, 64, 256, 256
NCORES = 8
BPC = B // NCORES          # batches per core
P = BPC * C                # 128 partitions

SIDE = [0, 2, 3, 5, 6, 8]  # taps with kx != 1 (PE diag matmuls)
CENT = [1, 4, 7]           # taps with kx == 1 (DVE mul/add chain)


def build_nc(h=H, w=W, loop_reps=1, prelu_ok=True):
    """Build the per-core SPMD Bass module (shapes [BPC,C,h,w]).

    loop_reps>1 wraps the steady-state body in a hardware For_i loop
    (timing).  prelu_ok=False emits relu+stt instead of AF.Prelu so the
    kernel can run under CoreSim (which lacks Prelu)."""
    pw = w + 4                 # padded row width: 2 pad cols each side
    qr = min(64, h)            # data rows per quarter cache tile
    nq = h // qr
    qrows = qr + 2             # rows per quarter tile (1-row halo each side)
    qn = qrows * pw
    npx = h * w
    n_groups = h // 2
    gpq = qr // 2              # groups per quarter
    lr = 8                     # image rows per load slab
    spq = qr // lr             # slabs per quarter

    nc = bacc.Bacc(trn_type="TRN2")

    feat = nc.dram_tensor("feat", [BPC, C, h, w], f32, kind="ExternalInput")
    deg = nc.dram_tensor("deg", [BPC, C], f32, kind="ExternalInput")
    wq = nc.dram_tensor("wq", [C, C], f32, kind="ExternalInput")
    bq = nc.dram_tensor("bq", [C], f32, kind="ExternalInput")
    wk = nc.dram_tensor("wk", [C, C], f32, kind="ExternalInput")
    bk = nc.dram_tensor("bk", [C], f32, kind="ExternalInput")
    wv = nc.dram_tensor("wv", [C, C], f32, kind="ExternalInput")
    bv = nc.dram_tensor("bv", [C], f32, kind="ExternalInput")
    gamma = nc.dram_tensor("gamma", [1], f32, kind="ExternalInput")
    k_w1 = nc.dram_tensor("k_w1", [C, C], f32, kind="ExternalInput")
    k_w2 = nc.dram_tensor("k_w2", [C * 9, C], f32, kind="ExternalInput")
    conv_w = nc.dram_tensor("conv_w", [C, C], f32, kind="ExternalInput")
    conv_b = nc.dram_tensor("conv_b", [C], f32, kind="ExternalInput")
    ca_w1 = nc.dram_tensor("ca_w1", [C // 8, C], f32, kind="ExternalInput")
    ca_w2 = nc.dram_tensor("ca_w2", [C, C // 8], f32, kind="ExternalInput")
    out = nc.dram_tensor("out", [BPC, C, h, w], f32, kind="ExternalOutput")

    featv = feat[:, :, :, :].rearrange("b c h w -> (b c) (h w)")
    outv = out[:, :, :, :].rearrange("b c h w -> (b c) (h w)")

    with tile.TileContext(nc) as tc, contextlib.ExitStack() as ctx:
        sing = ctx.enter_context(tc.tile_pool(name="sing", bufs=1))
        work = ctx.enter_context(tc.tile_pool(name="work", bufs=3))
        dr = ctx.enter_context(tc.tile_pool(name="dr", bufs=1, space="DRAM"))
        ps_v = ctx.enter_context(tc.tile_pool(name="ps_v", bufs=1, space="PSUM"))
        ps_d = ctx.enter_context(tc.tile_pool(name="ps_d", bufs=3, space="PSUM"))
        ps_o = ctx.enter_context(tc.tile_pool(name="ps_o", bufs=2, space="PSUM"))
        stgp = ctx.enter_context(tc.tile_pool(name="stgp", bufs=2))

        def lrelu_act(out_ap, in_ap, tagname):
            """out = lrelu(in); one scalar op on HW (Prelu honors alpha)."""
            if prelu_ok:
                nc.scalar.activation(out=out_ap, in_=in_ap, func=AF.Prelu,
                                     bias=0.0, scale=1.0, alpha=0.1)
            else:
                tr = work.tile([128] + [d[1] for d in in_ap.ap[1:]], f32,
                               tag=f"lr{tagname}", name=f"lr{tagname}")
                nc.scalar.activation(out=tr[...], in_=in_ap, func=AF.Relu,
                                     bias=0.0, scale=0.9)
                nc.vector.scalar_tensor_tensor(out_ap, in_ap, 0.1, tr[...],
                                               op0=OP.mult, op1=OP.add)

        # ===== persistent tiles (prologue-initialized, live across body) =====
        ident_b = sing.tile([128, 128], bf16)
        make_identity(nc, ident_b[:, :])

        def load_T(src_dram, rows, cols, name):
            t = sing.tile([cols, rows], f32, tag=f"T{name}", name=f"T{name}")
            ap = bass.AP(tensor=src_dram[:, :].tensor, offset=0,
                         ap=[[1, cols], [cols, rows]])
            nc.sync.dma_start(out=t[:, :], in_=ap)
            return t

        def blkdiag(tsb, rows, cols, dtype=f32, name=""):
            blk = sing.tile([128, 128], dtype, tag=f"blk{name}",
                            name=f"blk{name}")
            nc.gpsimd.memset(blk[:, :], 0.0)
            nc.vector.tensor_copy(blk[0:rows, 0:cols], tsb[:, :])
            nc.sync.dma_start(out=blk[64:64 + rows, 64:64 + cols],
                              in_=tsb[:, :])
            return blk

        wqT = load_T(wq, 64, 64, "wq")
        wkT = load_T(wk, 64, 64, "wk")
        wvT = load_T(wv, 64, 64, "wv")
        k_w1T = load_T(k_w1, 64, 64, "kw1")
        conv_wT = load_T(conv_w, 64, 64, "cw")
        ca_w1T = load_T(ca_w1, 8, 64, "ca1")      # [64, 8]
        ca_w2T = load_T(ca_w2, 64, 8, "ca2")      # [8, 64]

        BQ = blkdiag(wqT, 64, 64, name="q")
        BK = blkdiag(wkT, 64, 64, name="k")
        BV = blkdiag(wvT, 64, 64, name="v")
        BW1 = blkdiag(k_w1T, 64, 64, name="w1")
        BA1 = blkdiag(ca_w1T, 64, 8, name="a1")
        BA2 = blkdiag(ca_w2T, 8, 64, name="a2")
        conv_wTb = sing.tile([64, 64], bf16)
        nc.vector.tensor_copy(conv_wTb[:, :], conv_wT[:, :])
        CB = blkdiag(conv_wTb, 64, 64, dtype=bf16, name="cw")

        BK2 = []
        for t in range(9):
            w2t = sing.tile([64, 64], f32, tag=f"w2T{t}", name=f"w2T{t}")
            ap = bass.AP(tensor=k_w2[:, :].tensor, offset=t * 64,
                         ap=[[1, 64], [9 * 64, 64]])
            nc.sync.dma_start(out=w2t[:, :], in_=ap)
            BK2.append(blkdiag(w2t, 64, 64, name=f"k2_{t}"))

        def bcast_c(src, name):
            t = sing.tile([128, 1], f32, tag=f"pc{name}", name=f"pc{name}")
            ap = bass.AP(tensor=src[:].tensor, offset=0,
                         ap=[[0, BPC], [1, C]])
            nc.gpsimd.dma_start(out=t[:, 0:1], in_=ap)
            return t

        bq_pc = bcast_c(bq, "bq")
        bk_pc = bcast_c(bk, "bk")
        bv_pc = bcast_c(bv, "bv")
        conv_b_pc = bcast_c(conv_b, "cb")
        gamma_pc = sing.tile([128, 1], f32)
        nc.gpsimd.dma_start(
            out=gamma_pc[:, 0:1],
            in_=bass.AP(tensor=gamma[:].tensor, offset=0,
                        ap=[[0, 128], [1, 1]]))
        deg_pc = sing.tile([128, 1], f32)
        nc.sync.dma_start(out=deg_pc[:, 0:1],
                          in_=deg[:, :].rearrange("b c -> (b c)")
                          .rearrange("(p one) -> p one", one=1))

        def vec_mm(blk_w, rhs_pc, name):
            p = ps_v.tile([128, 1], f32, tag="vec", name=f"vm{name}")
            nc.tensor.matmul(p[:, 0:1], blk_w[:, :], rhs_pc[:, 0:1],
                             start=True, stop=True)
            return p

        def lrelu_vec(psum_in, name):
            o = sing.tile([128, 1], f32, tag=f"lro{name}", name=f"lro{name}")
            lrelu_act(o[:, 0:1], psum_in[:, 0:1], name)
            return o

        # --------- channel attention (depends only on deg; prologue) --------
        a0 = vec_mm(BA1, deg_pc, "a0")
        t_pr = lrelu_vec(a0, "a0")
        a1 = vec_mm(BA2, t_pr, "a1")
        att_pc = sing.tile([128, 1], f32)
        nc.scalar.activation(out=att_pc[:, 0:1], in_=a1[:, 0:1],
                             func=AF.Sigmoid, bias=0.0, scale=1.0)
        ATTD = sing.tile([128, 128], bf16)
        nc.vector.tensor_scalar_mul(ATTD[:, :], ident_b[:, :], att_pc[:, 0:1])

        kk0 = vec_mm(BK, deg_pc, "kk")
        kk_pi = sing.tile([128, 1], f32)
        nc.scalar.activation(out=kk_pi[:, 0:1], in_=kk0[:, 0:1],
                             func=AF.Identity, bias=bk_pc[:, 0:1], scale=1.0)
        v0 = vec_mm(BV, deg_pc, "v")
        v_pi = sing.tile([128, 1], f32)
        nc.scalar.activation(out=v_pi[:, 0:1], in_=v0[:, 0:1],
                             func=AF.Identity, bias=bv_pc[:, 0:1], scale=1.0)

        def rep64(src_pc, name):
            d = dr.tile([128], f32, tag=f"dr{name}", name=f"dr{name}")
            nc.sync.dma_start(out=d[:], in_=src_pc[:, 0:1])
            rep = sing.tile([128, 64], f32, tag=f"rep{name}",
                            name=f"rep{name}")
            ap = bass.AP(tensor=d[:].tensor, offset=d[:].offset,
                         ap=[[64, BPC], [0, C], [1, C]])
            nc.sync.dma_start(out=rep[:, :], in_=ap)
            return rep

        kk_rep = rep64(kk_pi, "kk")
        v_rep = rep64(v_pi, "v")

        # --------- quarter caches + pad memsets (prologue only) -------------
        quarts = []
        for q in range(nq):
            cq = sing.tile([128, qn], bf16, tag=f"cq{q}", name=f"cq{q}")
            quarts.append(cq)
            cqv = cq[:, :].rearrange("p (r z) -> p r z", z=pw)
            nc.gpsimd.memset(cqv[:, :, 0:2], 0.0)
            nc.gpsimd.memset(cqv[:, :, w + 2:w + 4], 0.0)
            if q == 0:
                nc.gpsimd.memset(cq[:, 0:pw], 0.0)
            if q == nq - 1:
                nc.gpsimd.memset(cq[:, (qrows - 1) * pw:qn], 0.0)
        partials = sing.tile([128, nq * spq], f32)

        def refill_quarter(q):
            """DMA + Copy + row sums for quarter q (8-row slabs + halos)."""
            cq = quarts[q]
            for i in range(spq):
                r_img = q * qr + i * lr
                dst = cq[:, (1 + i * lr) * pw:(1 + (i + 1) * lr) * pw] \
                    .rearrange("p (r z) -> p r z", z=pw)[:, :, 2:2 + w]
                stg = stgp.tile([128, lr * w], f32, tag="stg", name="stg")
                nc.sync.dma_start(out=stg[:, :],
                                  in_=featv[:, r_img * w:(r_img + lr) * w])
                nc.scalar.activation(
                    out=dst, in_=stg[:, :].rearrange("p (r z) -> p r z", z=w),
                    func=AF.Copy, bias=0.0, scale=1.0)
                nc.vector.reduce_sum(partials[:, q * spq + i:q * spq + i + 1],
                                     stg[:, :], axis=mybir.AxisListType.X)
            for (row_img, row_tile) in ((q * qr - 1, 0),
                                        ((q + 1) * qr, qrows - 1)):
                if row_img < 0 or row_img >= h:
                    continue
                hst = stgp.tile([128, w], f32, tag="hst", name="hst")
                nc.sync.dma_start(out=hst[:, :],
                                  in_=featv[:, row_img * w:(row_img + 1) * w])
                nc.scalar.activation(
                    out=cq[:, row_tile * pw + 2:row_tile * pw + 2 + w],
                    in_=hst[:, :], func=AF.Copy, bias=0.0, scale=1.0)

        # prologue load of all quarters
        for q in range(nq):
            refill_quarter(q)

        def win(g, idx):
            """[128, 2, w] window for tap idx at group g."""
            q = g // gpq
            rho = 2 * (g - q * gpq) + 1       # tile-local padded row
            ky, kx = idx // 3, idx % 3
            cap = quarts[q][:, :]
            base = (rho + ky - 1) * pw + 2 + (kx - 1)
            return bass.AP(tensor=cap.tensor, offset=cap.offset + base,
                           ap=[list(cap.ap[0]), [pw, 2], [1, w]])

        def body():
            # ---- CMA + kernel-predictor chain (uses partials from the
            # previous refill wave) ----
            feat_ave = sing.tile([128, 1], f32, tag="fave", name="fave")
            nc.vector.reduce_sum(feat_ave[:, 0:1], partials[:, :],
                                 axis=mybir.AxisListType.X)
            nc.vector.tensor_scalar_mul(feat_ave[:, 0:1], feat_ave[:, 0:1],
                                        1.0 / npx)
            q0 = vec_mm(BQ, feat_ave, "q")
            q_pj = sing.tile([128, 1], f32, tag="qpj", name="qpj")
            nc.scalar.activation(out=q_pj[:, 0:1], in_=q0[:, 0:1],
                                 func=AF.Identity, bias=bq_pc[:, 0:1],
                                 scale=1.0)
            energy = sing.tile([128, C], f32, tag="energy", name="energy")
            nc.vector.tensor_scalar_mul(energy[:, :], kk_rep[:, :],
                                        q_pj[:, 0:1])
            ee = sing.tile([128, C], f32, tag="ee", name="ee")
            nc.scalar.activation(out=ee[:, :], in_=energy[:, :], func=AF.Exp,
                                 bias=0.0, scale=1.0)
            es = sing.tile([128, 1], f32, tag="es", name="es")
            nc.vector.reduce_sum(es[:, 0:1], ee[:, :],
                                 axis=mybir.AxisListType.X)
            erc = sing.tile([128, 1], f32, tag="erc", name="erc")
            nc.vector.reciprocal(erc[:, 0:1], es[:, 0:1])
            attn = sing.tile([128, C], f32, tag="attn", name="attn")
            nc.vector.tensor_scalar_mul(attn[:, :], ee[:, :], erc[:, 0:1])
            prod = sing.tile([128, C], f32, tag="prodt", name="prodt")
            nc.vector.tensor_mul(prod[:, :], attn[:, :], v_rep[:, :])
            cma = sing.tile([128, 1], f32, tag="cma", name="cma")
            nc.vector.reduce_sum(cma[:, 0:1], prod[:, :],
                                 axis=mybir.AxisListType.X)
            emb = sing.tile([128, 1], f32, tag="emb", name="emb")
            nc.vector.scalar_tensor_tensor(emb[:, 0:1], cma[:, 0:1],
                                           gamma_pc[:, 0:1], deg_pc[:, 0:1],
                                           op0=OP.mult, op1=OP.add)
            hid0 = vec_mm(BW1, emb, "hid")
            hid_pc = lrelu_vec(hid0, "hid")
            ktp = ps_v.tile([128, 9], f32, tag="ktap", name="ktp")
            for t in range(9):
                nc.tensor.matmul(ktp[:, t:t + 1], BK2[t][:, :],
                                 hid_pc[:, 0:1], start=True, stop=True)
            k_tap = sing.tile([128, 9], f32, tag="ktapsb", name="k_tap")
            nc.scalar.copy(k_tap[:, :], ktp[:, :])
            diags = {}
            for t in SIDE:
                dg = sing.tile([128, 128], bf16, tag=f"diag{t}",
                               name=f"diag{t}")
                nc.vector.tensor_scalar_mul(dg[:, :], ident_b[:, :],
                                            k_tap[:, t:t + 1])
                diags[t] = dg

            # ---- main loop with interleaved next-wave refills ----
            for g in range(n_groups):
                a0t = work.tile([128, 2, w], bf16, tag="a0t", name="a0t")
                nc.vector.tensor_scalar_mul(a0t[:, :, :], win(g, CENT[0]),
                                            k_tap[:, CENT[0]:CENT[0] + 1])
                a1t = work.tile([128, 2, w], bf16, tag="a1t", name="a1t")
                nc.vector.tensor_scalar_mul(a1t[:, :, :], win(g, CENT[1]),
                                            k_tap[:, CENT[1]:CENT[1] + 1])
                a2t = work.tile([128, 2, w], bf16, tag="a2t", name="a2t")
                nc.vector.tensor_scalar_mul(a2t[:, :, :], win(g, CENT[2]),
                                            k_tap[:, CENT[2]:CENT[2] + 1])
                s1 = work.tile([128, 2, w], bf16, tag="s1", name="s1")
                nc.vector.tensor_add(s1[:, :, :], a0t[:, :, :], a1t[:, :, :])
                acc = work.tile([128, 2, w], bf16, tag="acc", name="acc")
                nc.vector.tensor_add(acc[:, :, :], s1[:, :, :], a2t[:, :, :])
                pd = ps_d.tile([128, 2 * w], f32, tag="pd", name="pd")
                for j, idx in enumerate(SIDE):
                    nc.tensor.matmul(pd[:, :], diags[idx][:, :], win(g, idx),
                                     start=(j == 0), stop=False)
                nc.tensor.matmul(pd[:, :], ident_b[:, :],
                                 acc[:, :, :].rearrange("p a b -> p (a b)"),
                                 start=False, stop=True)
                y = work.tile([128, 2 * w], bf16, tag="y", name="y")
                lrelu_act(y[:, :], pd[:, :], "y")
                po = ps_o.tile([128, 2 * w], f32, tag="po", name="po")
                nc.tensor.matmul(po[:, :], CB[:, :], y[:, :],
                                 start=True, stop=False)
                nc.tensor.matmul(po[:, :], ATTD[:, :], win(g, 4),
                                 start=False, stop=True)
                out_s = work.tile([128, 2 * w], f32, tag="out_s",
                                  name="out_s")
                nc.scalar.activation(out=out_s[:, :], in_=po[:, :],
                                     func=AF.Identity, bias=conv_b_pc[:, 0:1],
                                     scale=1.0)
                nc.sync.dma_start(out=outv[:, g * 2 * w:(g + 1) * 2 * w],
                                  in_=out_s[:, :])
                # refill quarter q right after its last group is consumed
                if (g + 1) % gpq == 0:
                    refill_quarter(g // gpq)

        if loop_reps > 1:
            with tc.For_i(0, loop_reps, 1):
                body()
        else:
            body()

    nc.finalize()
    return nc


_NC_CACHE = {}


def _get_nc(h, w):
    if (h, w) not in _NC_CACHE:
        _NC_CACHE[(h, w)] = build_nc(h, w)
    return _NC_CACHE[(h, w)]


def kernel(**inputs):
    from concourse.bass_utils import run_bass_kernel_spmd

    feat = np.ascontiguousarray(inputs["feat"], dtype=np.float32)
    deg = np.ascontiguousarray(inputs["deg"], dtype=np.float32)
    b, c, h, w = feat.shape
    nc = _get_nc(h, w)

    shared = {k: np.ascontiguousarray(np.asarray(v), dtype=np.float32)
              for k, v in inputs.items() if k not in ("feat", "deg")}
    in_maps = []
    for k in range(NCORES):
        m = dict(shared)
        m["feat"] = feat[k * BPC:(k + 1) * BPC]
        m["deg"] = deg[k * BPC:(k + 1) * BPC]
        in_maps.append(m)

    res = run_bass_kernel_spmd(nc, in_maps, core_ids=list(range(NCORES)))
    return np.concatenate([r["out"] for r in res.results], axis=0)
